# revision 43
# baseline (speedup 1.0000x reference)
"""Trainium2 Bass kernel for MADE autoregressive sampling (rsample).

Structure exploited (degrees mh = arange(512)%63 + 1, sorted):
  - sorted hidden units split exactly into 4 partition groups of 128
    (degrees 1-15 | 16-31 | 32-47 | 48-63);
  - every h1/h2 unit is final once z_{deg-1} is known -> computed exactly
    once, at step == its degree;
  - masked output weights let per-degree h2 contributions accumulate into a
    persistent PSUM accumulator without corrupting already-read outputs.

v3: the per-step critical chain contains NO DMA.  The z scatter into
unit-land (zT) has ~4us latency, so the L1 z-matmul only reads zT rows
<= i-4 (scattered >= 3 steps ago); the last 3 z columns enter p1 via
batch-land outer products (DVE) that are transpose-ACCUMULATED into the
p1 PSUM group (PE, float32r).  ctx_h + b1 ride along in the same
accumulation (ctxB, batch-land, computed once at init), which removes the
per-step ctx staging DMAs entirely.  L3 matmuls write only the not-yet-
read output columns (>= i), mu outputs are read from the PSUM accumulator
once at the end.  All matmuls with output free >= 256 run as float32r
(1 cycle/row vs 4 for fp32).

Layouts (per core, batch shard BS=1024, halves of 512):
  unit-land  : features on partitions, batch on free dim.  Column j of half h
               is batch row r = h*512 + (j%4)*128 + j//4.
  batch-land : output accumulators / z / eps / outputs keep batch on
               partitions (128) x 4 chunks side-by-side in the free dim.
Compute-engine APs keep in/out partition bases equal; every partition-
crossing move is a DMA (z scatter on the Pool queue, h1 assembly on SP).
"""

import numpy as np

B, D, CTX, H = 8192, 256 // 4, 256, 512
NCORES = 8
BS = B // NCORES   # 1024 rows per core
NH = BS // 2       # 512 per half (fp32 matmul moving-operand max)
MMAX = 9           # max units per degree
KC = 1             # z lag: zmm covers rows <= i-2; col i-1 completes in SBUF


def _structure():
    m0 = np.arange(1, D + 1)
    mh = (np.arange(H) % (D - 1)) + 1
    M1 = (mh[:, None] >= m0[None, :]).astype(np.float32)   # (H, D)
    M2 = (mh[:, None] >= mh[None, :]).astype(np.float32)   # (H, H)
    mo = np.concatenate([m0, m0])
    Mo = (mo[:, None] > mh[None, :]).astype(np.float32)    # (2D, H)
    perm = np.argsort(mh, kind="stable")
    smh = mh[perm]
    S = np.zeros(D, np.int64)
    E = np.zeros(D, np.int64)
    for i in range(1, D):
        S[i] = np.searchsorted(smh, i, side="left")
        E[i] = np.searchsorted(smh, i, side="right")
    return M1, M2, Mo, perm, S, E


_M1, _M2, _Mo, _PERM, _S, _E = _structure()
# unit-land column j <-> shard row j (chunk-major: j = c*128 + p)


def _host_weights(W1, b1, Wc, W2, b2, Wo, bo):
    W1m = (W1 * _M1).T[:, _PERM]                     # (64, 512)
    W2m = ((W2 * _M2).T)[_PERM][:, _PERM]            # (512, 512)
    Wom = ((Wo * _Mo).T)[_PERM, :]                   # (512, 128)
    Wcs = Wc[_PERM]                                  # (512, 256)
    b1s_ = b1[_PERM]
    b2s_ = b2[_PERM]

    NQ = 16                         # quads of 4 degrees at 32-row offsets
    # womp rows sit at 32*((i-1)%4) to match the h2 quad-row layout
    womp = np.zeros((128, D * 2 * D), np.float32)
    for i in range(1, D):
        s, e = int(_S[i]), int(_E[i])
        o = 32 * ((i - 1) % 4)
        womp[o:o + e - s, i * 2 * D:(i + 1) * 2 * D] = Wom[s:e, :]

    def quad_cols(q):
        """(col -> sorted-unit) gather for quad q; -1 = padding."""
        idx = np.full(128, -1, np.int64)
        for sd in range(4):
            d = 4 * q + 1 + sd
            if d < D:
                s, e = int(_S[d]), int(_E[d])
                idx[32 * sd:32 * sd + e - s] = np.arange(s, e)
        return idx

    w2q = np.zeros((H, NQ * 128), np.float32)       # group rows x quad cols
    w2src = np.zeros((MMAX, D * 128), np.float32)   # per-deg rows x quad cols
    b2q = np.zeros((128, NQ), np.float32)
    for q in range(NQ):
        idx = quad_cols(q)
        val = np.where(idx >= 0, 1.0, 0.0)
        w2q[:, q * 128:(q + 1) * 128] = W2m[:, np.maximum(idx, 0)] * val
        b2q[:, q] = b2s_[np.maximum(idx, 0)] * val
    w2fwd = np.zeros((MMAX, NQ * 128), np.float32)  # slot-3 deg -> next quad
    for d in range(1, D):
        s, e = int(_S[d]), int(_E[d])
        idx = quad_cols((d - 1) // 4)
        val = np.where(idx >= 0, 1.0, 0.0)
        w2src[:e - s, d * 128:(d + 1) * 128] = (
            W2m[s:e][:, np.maximum(idx, 0)] * val)
        if d % 4 == 0 and d // 4 < NQ:
            idx = quad_cols(d // 4)
            val = np.where(idx >= 0, 1.0, 0.0)
            w2fwd[:e - s, (d // 4) * 128:(d // 4 + 1) * 128] = (
                W2m[s:e][:, np.maximum(idx, 0)] * val)

    # completion weights, replicated on all 128 partitions:
    #   w1cF[*, i, u] = W1m[i-1, S[i]+u]   (fresh z col, on the chain)
    w1f = np.zeros((D, MMAX), np.float32)
    for i in range(1, D):
        s, e = int(_S[i]), int(_E[i])
        w1f[i, :e - s] = W1m[i - 1, s:e]
    w1cF = np.broadcast_to(w1f.reshape(1, -1), (128, D * MMAX))

    W1mp = np.zeros((D, H + MMAX), np.float32)
    W1mp[:, :H] = W1m
    import ml_dtypes
    bf = ml_dtypes.bfloat16
    return {
        "w1m": W1mp.astype(bf),
        "w1z": np.zeros((1, MMAX), bf),
        "w2q": w2q.astype(bf),
        "w2src": w2src.astype(bf),
        "w2fwd": w2fwd.astype(bf),
        "wct": np.ascontiguousarray(Wcs.T).astype(bf),         # (256, 512)
        "b1r": np.ascontiguousarray(b1s_[None, :]).astype(bf),  # (1, 512)
        "w1cF": np.ascontiguousarray(w1cF, np.float32),
        "ident": np.eye(128, dtype=np.float32),
        "womp": womp.astype(bf),
        "b2q": b2q,
        "bo2": np.ascontiguousarray(bo[None, :], np.float32),  # (1, 128)
        "ones": np.ones((1, 128), np.float32),
        "onesb": np.ones((1, 128), bf),
        "zz": np.zeros((D, BS), bf),
    }


_NC_CACHE = {}


def _build():
    if "nc" in _NC_CACHE:
        return _NC_CACHE["nc"]
    from contextlib import ExitStack

    import concourse.mybir as mybir
    import concourse.tile as tile
    from concourse import bacc

    f32 = mybir.dt.float32
    f32r = mybir.dt.float32r
    bf16 = mybir.dt.bfloat16
    AF = mybir.ActivationFunctionType
    OP = mybir.AluOpType
    AX = mybir.AxisListType

    def r(ap):
        # fp32 operands pass through: the fast matmuls all run in bf16
        # (float32r needs producer-side rounding the BIR verifier enforces)
        return ap

    # All ACT funcs used here (exp, ln, relu, identity) live in the
    # "natural_log_exp_and_others" table.  The greedy table-selection pass
    # otherwise ping-pongs exp->ln between single-func tables, inserting
    # ~256 table loads.  Keep dict order (index == act_func_set_id) but
    # blank every other table so selection sticks to the combined one.
    import concourse.bacc as bacc_mod
    _orig_tables = bacc_mod.get_activation_tables

    def _one_table(arch):
        tabs = _orig_tables(arch)
        return {k: (v if k == "natural_log_exp_and_others" else set())
                for k, v in tabs.items()}

    bacc_mod.get_activation_tables = _one_table

    nc = bacc.Bacc("TRN2", target_bir_lowering=False)

    ctxT_d = nc.dram_tensor("ctxT", [CTX, BS], bf16, kind="ExternalInput")
    epsB_d = nc.dram_tensor("epsB", [128, 8 * D], f32, kind="ExternalInput")
    w1m_d = nc.dram_tensor("w1m", [D, H + MMAX], bf16, kind="ExternalInput")
    w1z_d = nc.dram_tensor("w1z", [1, MMAX], bf16, kind="ExternalInput")
    w2q_d = nc.dram_tensor("w2q", [H, 16 * 128], bf16, kind="ExternalInput")
    wct_d = nc.dram_tensor("wct", [CTX, H], bf16, kind="ExternalInput")
    b1r_d = nc.dram_tensor("b1r", [1, H], bf16, kind="ExternalInput")
    w1f_d = nc.dram_tensor("w1cF", [128, D * MMAX], f32, kind="ExternalInput")
    id_d = nc.dram_tensor("ident", [128, 128], f32, kind="ExternalInput")
    w2s_d = nc.dram_tensor("w2src", [MMAX, D * 128], bf16, kind="ExternalInput")
    w2f_d = nc.dram_tensor("w2fwd", [MMAX, 16 * 128], bf16, kind="ExternalInput")
    womp_d = nc.dram_tensor("womp", [128, D * 2 * D], bf16, kind="ExternalInput")
    b2q_d = nc.dram_tensor("b2q", [128, 16], f32, kind="ExternalInput")
    bo2_d = nc.dram_tensor("bo2", [1, 2 * D], f32, kind="ExternalInput")
    ones_d = nc.dram_tensor("ones", [1, 128], f32, kind="ExternalInput")
    onesb_d = nc.dram_tensor("onesb", [1, 128], bf16, kind="ExternalInput")
    zz_d = nc.dram_tensor("zz", [D, BS], bf16, kind="ExternalInput")

    # outputs, batch-major (BS, D); rows r = h*512 + ch*128 + p
    zo_d = nc.dram_tensor("zo", [BS, D], f32, kind="ExternalOutput")
    mo_d = nc.dram_tensor("mo", [BS, D], f32, kind="ExternalOutput")
    so_d = nc.dram_tensor("so", [BS, D], f32, kind="ExternalOutput")

    with tile.TileContext(nc) as tc, ExitStack() as ctx:
        const = ctx.enter_context(tc.tile_pool(name="const", bufs=1))
        work = ctx.enter_context(tc.tile_pool(name="work", bufs=4))
        pout = ctx.enter_context(tc.tile_pool(name="pout", bufs=1, space="PSUM"))
        pscr = ctx.enter_context(tc.tile_pool(name="pscr", bufs=2, space="PSUM"))
        pzt = ctx.enter_context(tc.tile_pool(name="pzt", bufs=2, space="PSUM"))

        # ---- constants / state ----
        w1m = const.tile([D, H + MMAX], bf16)
        nc.sync.dma_start(w1m[:, :], w1m_d[:, :])
        w1z = const.tile([1, MMAX], bf16)
        nc.sync.dma_start(w1z[:, :], w1z_d[:, :])
        w2q = [const.tile([128, 16 * 128], bf16, name=f"w2q{g}") for g in range(4)]
        for g in range(4):
            nc.sync.dma_start(w2q[g][:, :], w2q_d[g * 128:(g + 1) * 128, :])
        wct = [const.tile([128, H], bf16, name=f"wct{k}") for k in range(2)]
        for k in range(2):
            nc.sync.dma_start(wct[k][:, :], wct_d[k * 128:(k + 1) * 128, :])
        ctxT = [const.tile([128, BS], bf16, name=f"ctxTs{k}") for k in range(2)]
        for k in range(2):
            nc.sync.dma_start(ctxT[k][:, :], ctxT_d[k * 128:(k + 1) * 128, :])
        b1r = const.tile([1, H], bf16)
        nc.sync.dma_start(b1r[:, :], b1r_d[:, :])
        w1cF = const.tile([128, D * MMAX], f32)
        nc.sync.dma_start(w1cF[:, :], w1f_d[:, :])
        ident = const.tile([128, 128], f32)
        nc.sync.dma_start(ident[:, :], id_d[:, :])
        w2src = const.tile([MMAX, D * 128], bf16)
        nc.sync.dma_start(w2src[:, :], w2s_d[:, :])
        w2fwd = const.tile([MMAX, 16 * 128], bf16)
        nc.sync.dma_start(w2fwd[:, :], w2f_d[:, :])
        womp = const.tile([128, D * 2 * D], bf16)
        nc.sync.dma_start(womp[:, :], womp_d[:, :])
        b2q = const.tile([128, 16], f32)
        nc.sync.dma_start(b2q[:, :], b2q_d[:, :])
        bo2 = const.tile([1, 2 * D], f32)
        nc.sync.dma_start(bo2[:, :], bo2_d[:, :])
        ones = const.tile([1, 128], f32)
        nc.sync.dma_start(ones[:, :], ones_d[:, :])
        onesb = const.tile([1, 128], bf16)
        nc.sync.dma_start(onesb[:, :], onesb_d[:, :])
        epsB = const.tile([128, 8 * D], f32)
        nc.sync.dma_start(epsB[:, :], epsB_d[:, :])
        zT = const.tile([D, BS], bf16)
        nc.sync.dma_start(zT[:, :], zz_d[:, :])

        h1g = [const.tile([128, BS], bf16, name=f"h1g{g}") for g in range(4)]
        scB = const.tile([128, 8 * D], f32)
        zB = const.tile([128, 8 * D], f32)
        ctxB = const.tile([128, 8 * (H + MMAX)], f32)
        nc.vector.memset(zB[:, :], 0.0)
        nc.vector.memset(ctxB[:, :], 0.0)

        # persistent transposed output accumulators: [batch 128, 4ch x 128 out]
        outp = [pout.tile([128, 4 * 128], f32, name=f"outp{h}") for h in range(2)]

        def ov(h, ch):            # (128, 128) chunk view of the accumulator
            return outp[h][:, ch * 128:(ch + 1) * 128]

        def ocol(h, o):           # (128, 4) strided column view, output o
            return outp[h][:, :].rearrange("p (c o) -> p c o", c=4)[:, :, o]

        def bcolB(t, i):          # (128, 8) strided column, both halves
            return t[:, :].rearrange("p (g d) -> p g d", g=8)[:, :, i]

        def bcol(t, h, i):        # (128, 4) strided column of half h
            return t[:, h * 4 * D:(h + 1) * 4 * D].rearrange(
                "p (c d) -> p c d", c=4)[:, :, i]

        # ---- init: bias rows of the output accumulator ----
        for h in range(2):
            for ch in range(4):
                nc.tensor.matmul(ov(h, ch), ones[0:1, :], bo2[0:1, :],
                                 start=True, stop=True)

        # ---- init: batch-land ctx projection + b1 (ctxB[h][p, c*H+u]) ----
        for h in range(2):
            for c in range(4):
                pc = pscr.tile([128, H], f32, tag="p2", name=f"pc{h}_{c}")
                for k in range(2):
                    cstat = ctxT[k][:, h * NH + c * 128:h * NH + (c + 1) * 128]
                    nc.tensor.matmul(pc[:, :], r(cstat),
                                     r(wct[k][:, :]), start=(k == 0),
                                     stop=False)
                nc.tensor.matmul(pc[:, :], onesb[0:1, :],
                                 b1r[0:1, :], start=False, stop=True)
                gc = h * 4 + c
                nc.scalar.activation(
                    ctxB[:, gc * (H + MMAX):gc * (H + MMAX) + H], pc[:, :],
                    AF.Identity)

        def extract(i):
            # softplus(x) = ln(exp(x) + 1)  (no softplus table on HW)
            for h in range(2):
                nc.scalar.activation(bcol(scB, h, i), ocol(h, D + i), AF.Exp)
                nc.scalar.activation(bcol(scB, h, i), bcol(scB, h, i),
                                     AF.Ln, bias=1.0)
                nc.vector.tensor_tensor(bcol(zB, h, i), bcol(scB, h, i),
                                        bcol(epsB, h, i), OP.mult)
                nc.vector.tensor_tensor(bcol(zB, h, i), bcol(zB, h, i),
                                        ocol(h, i), OP.add)
            # z_i to unit-land: PE transpose (128,8)->(8,128), then one
            # contiguous PSUM->SBUF DMA into zT row i (SP queue).
            # ~4us total latency, consumed only at step i+KC+1.
            with tc.high_priority(offset=-300):
                ztp = pzt.tile([8, 128], f32, tag="ztp", name=f"ztp{i}")
                nc.tensor.matmul(r(ztp[:, :]), r(bcolB(zB, i)),
                                 r(ident[:, :]), is_transpose=True,
                                 start=True, stop=True)
                zsb = work.tile([8, 128], bf16, tag="zsb", name=f"zsb{i}")
                nc.vector.tensor_copy(zsb[:, :], ztp[:, :])
                nc.sync.dma_start(
                    zT[i:i + 1, :].rearrange("a (g p) -> a g p", g=8),
                    zsb[:, :])

        extract(0)

        h1n_t = {}                 # live per-degree h1 tiles
        p2q = [None, None]         # current quad's L2 accumulator per half
        h2q = [None, None]         # current quad's relu'd h2 rows per half
        for i in range(1, D):
            s, e = int(_S[i]), int(_E[i])
            m = e - s
            g_, r0_ = s // 128, s % 128
            q, sd = (i - 1) // 4, (i - 1) % 4
            ps = int(_S[i - 1])    # h1g rows < ps are >= 2 steps old
            nz = max(0, i - KC)    # zT rows entering via the matmul
            j0 = max(0, i - KC)    # first z col entering via completion
            # --- completions, batch-land, both halves per op.  The old
            #     z col (i-2) + ctx are off the chain; only the fresh col
            #     (i-1) product and one add separate z_{i-1} from p1. ---
            def colv(t, j):       # (128, 8, MMAX) broadcast of col j
                return t[:, :].rearrange("p (g d) -> p g d", g=8)[
                    :, :, j:j + 1].broadcast_to((128, 8, MMAX))

            def wv9(t):           # (128, 8, MMAX) broadcast of a weight row
                return t[:, MMAX * i:MMAX * (i + 1)].rearrange(
                    "p (o u) -> p o u", o=1).broadcast_to((128, 8, MMAX))

            ctxv = ctxB[:, :].rearrange(
                "p (g u) -> p g u", g=8)[:, :, s:s + MMAX]
            cred = work.tile([128, 8 * MMAX], f32, tag="cred",
                             name=f"cred_{i}")
            credv = cred[:, :].rearrange("p (g u) -> p g u", g=8)
            for h in range(2):
                hs = slice(4 * h, 4 * (h + 1))
                nc.vector.tensor_tensor(credv[:, hs, :],
                                        colv(zB, i - 1)[:, hs, :],
                                        wv9(w1cF)[:, hs, :], OP.mult)
                nc.vector.tensor_tensor(credv[:, hs, :], credv[:, hs, :],
                                        ctxv[:, hs, :], OP.add)
            for h in range(2):
                cs = slice(h * NH, (h + 1) * NH)
                # --- L1: lagged z-matmul (rows <= i-KC-1), off the chain ---
                p1 = pscr.tile([MMAX, NH], f32, tag="p1", name=f"p1_{i}_{h}")
                if nz > 0:
                    nc.tensor.matmul(p1[0:MMAX, :], r(w1m[0:nz, s:s + MMAX]),
                                     r(zT[0:nz, cs]), start=True, stop=False)
                else:
                    nc.tensor.matmul(p1[0:MMAX, :], r(w1z[0:1, 0:MMAX]),
                                     r(zT[0:1, cs]), start=True, stop=False)
                # --- transpose-accumulate completions into the p1 group ---
                for c in range(4):
                    nc.tensor.matmul(
                        r(p1[0:MMAX, 128 * c:128 * (c + 1)]),
                        r(credv[:, h * 4 + c, :]), r(ident[:, :]),
                        is_transpose=True, start=False, stop=(c == 3),
                        skip_group_check=(c != 3))
                # --- h1 relu (Pool, bf16 out) + assembly into h1g (Pool q) ---
                h1n = work.tile([MMAX, NH], bf16, tag="h1n", name=f"h1n_{i}_{h}")
                if h == 0:
                    nc.scalar.activation(h1n[0:m, :], p1[0:m, :], AF.Relu)
                else:
                    nc.vector.tensor_scalar_max(h1n[0:m, :], p1[0:m, :], 0.0)
                with tc.high_priority(offset=-300):
                    eng = nc.sync if h == 0 else nc.gpsimd
                    eng.dma_start(h1g[g_][r0_:r0_ + m, cs], h1n[0:m, :])
                h1n_t[(i, h)] = h1n
                # --- L2, quad-grained: at each quad start accumulate all
                #     h1 degs <= i into the (128, 512) quad psum; slots 1-3
                #     add only their own degree's contribution. ---
                qsl = slice(q * 128, (q + 1) * 128)
                if sd == 0:
                    p2 = pscr.tile([128, NH], f32, tag="p2", name=f"p2_{i}_{h}")
                    p2q[h] = p2
                    h2q[h] = work.tile([128, NH], bf16, tag="h2n",
                                       name=f"h2n_{i}_{h}")
                    ngg = ps // 128
                    nmm = ngg + (1 if ps % 128 else 0) + (1 if i >= 2 else 0) + 1
                    kk = 0
                    for gg in range(ngg):
                        nc.tensor.matmul(p2[:, :], w2q[gg][:, qsl],
                                         h1g[gg][:, cs],
                                         start=(kk == 0), stop=False)
                        kk += 1
                    if ps % 128:
                        nc.tensor.matmul(p2[:, :], w2q[ngg][0:ps % 128, qsl],
                                         h1g[ngg][0:ps % 128, cs],
                                         start=(kk == 0), stop=False)
                        kk += 1
                    if i >= 2:
                        pm = int(_E[i - 1]) - ps
                        nc.tensor.matmul(p2[:, :],
                                         w2fwd[0:pm, q * 128:(q + 1) * 128],
                                         h1n_t[(i - 1, h)][0:pm, :],
                                         start=(kk == 0), stop=False)
                        kk += 1
                    for cb in range(2):
                        csl = slice(cb * NH // 2, (cb + 1) * NH // 2)
                        nc.tensor.matmul(p2[:, csl],
                                         w2src[0:m, i * 128:(i + 1) * 128],
                                         h1n[0:m, csl],
                                         start=(kk == 0 and cb == 0),
                                         stop=(cb == 1))
                else:
                    for cb in range(2):
                        csl = slice(cb * NH // 2, (cb + 1) * NH // 2)
                        nc.tensor.matmul(p2q[h][:, csl],
                                         w2src[0:m, i * 128:(i + 1) * 128],
                                         h1n[0:m, csl], start=False, stop=True,
                                         skip_group_check=True)
                # --- h2 relu of this degree's 32-aligned quad rows,
                #     split across ACT and DVE to shorten the chain ---
                h2n = h2q[h]
                o_ = 32 * sd
                if h == 0:
                    nc.scalar.activation(h2n[o_:o_ + m, :],
                                         p2q[h][o_:o_ + m, :], AF.Relu,
                                         bias=b2q[o_:o_ + m, q:q + 1])
                else:
                    nc.vector.tensor_scalar(h2n[o_:o_ + m, :],
                                            p2q[h][o_:o_ + m, :],
                                            b2q[o_:o_ + m, q:q + 1], 0.0,
                                            OP.add, OP.max)
                # --- L3: masked accumulate (cols i..63, D+i..2D-1 only) ---
                wb = womp[32 * sd:32 * sd + m, i * 2 * D:(i + 1) * 2 * D]
                for ch in range(4):
                    for t in range(2):
                        nc.tensor.matmul(
                            ov(h, ch)[:, t * D + i:(t + 1) * D],
                            h2n[32 * sd:32 * sd + m, ch * 128:(ch + 1) * 128],
                            wb[:, t * D + i:(t + 1) * D],
                            start=False, stop=True, skip_group_check=True,
                            tile_position=(32 * sd, 0))
            extract(i)

        # ---- outputs (batch-major rows r = h*512 + ch*128 + p) ----
        for h in range(2):
            dst = slice(h * NH, (h + 1) * NH)
            for name_d, t in ((zo_d, zB), (so_d, scB)):
                nc.sync.dma_start(
                    name_d[dst, :].rearrange("(c p) d -> p c d", c=4),
                    t[:, h * 4 * D:(h + 1) * 4 * D].rearrange(
                        "p (c d) -> p c d", c=4))
            # mu out of the PSUM accumulator via one SBUF bounce
            muB = work.tile([128, 4 * D], f32, tag="muB", name=f"muB_{h}")
            nc.scalar.activation(
                muB[:, :].rearrange("p (c d) -> p c d", c=4),
                outp[h][:, :].rearrange("p (c o) -> p c o", c=4)[:, :, 0:D],
                AF.Identity)
            nc.sync.dma_start(
                mo_d[dst, :].rearrange("(c p) d -> p c d", c=4),
                muB[:, :].rearrange("p (c d) -> p c d", c=4))

    nc.compile()
    _NC_CACHE["nc"] = nc
    return nc


def make_in_maps(context, eps, W1, b1, Wc, W2, b2, Wo, bo):
    context = np.asarray(context, np.float32)
    eps = np.asarray(eps, np.float32)
    wd = _host_weights(np.asarray(W1, np.float32), np.asarray(b1, np.float32),
                       np.asarray(Wc, np.float32), np.asarray(W2, np.float32),
                       np.asarray(b2, np.float32), np.asarray(Wo, np.float32),
                       np.asarray(bo, np.float32))

    in_maps = []
    for c in range(NCORES):
        sl = slice(c * BS, (c + 1) * BS)
        ctx_s = context[sl]                       # (1024, 256)
        eps_s = eps[sl]                           # (1024, 64)
        im = dict(wd)
        im["ctxT"] = np.ascontiguousarray(ctx_s.T).astype(
            wd["wct"].dtype)                                  # (256, 1024)
        im["epsB"] = np.ascontiguousarray(
            eps_s.reshape(2, 4, 128, D).transpose(2, 0, 1, 3).reshape(
                128, 8 * D))
        in_maps.append(im)
    return in_maps


def assemble_outputs(results):
    z = np.concatenate([r["zo"] for r in results], axis=0)
    mus = np.concatenate([r["mo"] for r in results], axis=0)
    scales = np.concatenate([r["so"] for r in results], axis=0)
    return z, mus, scales


def kernel(context, eps, W1, b1, Wc, W2, b2, Wo, bo, _trace=False):
    from concourse.bass_utils import run_bass_kernel_spmd

    in_maps = make_in_maps(context, eps, W1, b1, Wc, W2, b2, Wo, bo)
    nc = _build()
    res = run_bass_kernel_spmd(nc, in_maps, core_ids=list(range(NCORES)),
                               trace=_trace)
    if _trace:
        kernel.last_exec_time_ns = res.exec_time_ns
        kernel.last_results = res
    return assemble_outputs(res.results)


# revision 44
# speedup vs baseline: 261.5752x; 261.5752x over previous
"""Trainium2 Bass kernel for MADE autoregressive sampling (rsample).

Structure exploited (degrees mh = arange(512)%63 + 1, sorted):
  - sorted hidden units split exactly into 4 partition groups of 128
    (degrees 1-15 | 16-31 | 32-47 | 48-63);
  - every h1/h2 unit is final once z_{deg-1} is known -> computed exactly
    once, at step == its degree;
  - masked output weights let per-degree h2 contributions accumulate into a
    persistent PSUM accumulator without corrupting already-read outputs.

v3: the per-step critical chain contains NO DMA.  The z scatter into
unit-land (zT) has ~4us latency, so the L1 z-matmul only reads zT rows
<= i-4 (scattered >= 3 steps ago); the last 3 z columns enter p1 via
batch-land outer products (DVE) that are transpose-ACCUMULATED into the
p1 PSUM group (PE, float32r).  ctx_h + b1 ride along in the same
accumulation (ctxB, batch-land, computed once at init), which removes the
per-step ctx staging DMAs entirely.  L3 matmuls write only the not-yet-
read output columns (>= i), mu outputs are read from the PSUM accumulator
once at the end.  All matmuls with output free >= 256 run as float32r
(1 cycle/row vs 4 for fp32).

Layouts (per core, batch shard BS=1024, halves of 512):
  unit-land  : features on partitions, batch on free dim.  Column j of half h
               is batch row r = h*512 + (j%4)*128 + j//4.
  batch-land : output accumulators / z / eps / outputs keep batch on
               partitions (128) x 4 chunks side-by-side in the free dim.
Compute-engine APs keep in/out partition bases equal; every partition-
crossing move is a DMA (z scatter on the Pool queue, h1 assembly on SP).
"""

import numpy as np

B, D, CTX, H = 8192, 256 // 4, 256, 512
NCORES = 8
BS = B // NCORES   # 1024 rows per core
NH = BS // 2       # 512 per half (fp32 matmul moving-operand max)
MMAX = 9           # max units per degree
KC = 1             # z lag: zmm covers rows <= i-2; col i-1 completes in SBUF


def _structure():
    m0 = np.arange(1, D + 1)
    mh = (np.arange(H) % (D - 1)) + 1
    M1 = (mh[:, None] >= m0[None, :]).astype(np.float32)   # (H, D)
    M2 = (mh[:, None] >= mh[None, :]).astype(np.float32)   # (H, H)
    mo = np.concatenate([m0, m0])
    Mo = (mo[:, None] > mh[None, :]).astype(np.float32)    # (2D, H)
    perm = np.argsort(mh, kind="stable")
    smh = mh[perm]
    S = np.zeros(D, np.int64)
    E = np.zeros(D, np.int64)
    for i in range(1, D):
        S[i] = np.searchsorted(smh, i, side="left")
        E[i] = np.searchsorted(smh, i, side="right")
    return M1, M2, Mo, perm, S, E


_M1, _M2, _Mo, _PERM, _S, _E = _structure()
# unit-land column j <-> shard row j (chunk-major: j = c*128 + p)


def _host_weights(W1, b1, Wc, W2, b2, Wo, bo):
    W1m = (W1 * _M1).T[:, _PERM]                     # (64, 512)
    W2m = ((W2 * _M2).T)[_PERM][:, _PERM]            # (512, 512)
    Wom = ((Wo * _Mo).T)[_PERM, :]                   # (512, 128)
    Wcs = Wc[_PERM]                                  # (512, 256)
    b1s_ = b1[_PERM]
    b2s_ = b2[_PERM]

    NQ = 16                         # quads of 4 degrees at 32-row offsets
    # womp rows sit at 32*((i-1)%4) to match the h2 quad-row layout
    womp = np.zeros((128, D * 2 * D), np.float32)
    for i in range(1, D):
        s, e = int(_S[i]), int(_E[i])
        o = 32 * ((i - 1) % 4)
        womp[o:o + e - s, i * 2 * D:(i + 1) * 2 * D] = Wom[s:e, :]

    def quad_cols(q):
        """(col -> sorted-unit) gather for quad q; -1 = padding."""
        idx = np.full(128, -1, np.int64)
        for sd in range(4):
            d = 4 * q + 1 + sd
            if d < D:
                s, e = int(_S[d]), int(_E[d])
                idx[32 * sd:32 * sd + e - s] = np.arange(s, e)
        return idx

    w2q = np.zeros((H, NQ * 128), np.float32)       # group rows x quad cols
    w2src = np.zeros((MMAX, D * 128), np.float32)   # per-deg rows x quad cols
    b2q = np.zeros((128, NQ), np.float32)
    for q in range(NQ):
        idx = quad_cols(q)
        val = np.where(idx >= 0, 1.0, 0.0)
        w2q[:, q * 128:(q + 1) * 128] = W2m[:, np.maximum(idx, 0)] * val
        b2q[:, q] = b2s_[np.maximum(idx, 0)] * val
    w2fwd = np.zeros((MMAX, NQ * 128), np.float32)  # slot-3 deg -> next quad
    for d in range(1, D):
        s, e = int(_S[d]), int(_E[d])
        idx = quad_cols((d - 1) // 4)
        val = np.where(idx >= 0, 1.0, 0.0)
        w2src[:e - s, d * 128:(d + 1) * 128] = (
            W2m[s:e][:, np.maximum(idx, 0)] * val)
        if d % 4 == 0 and d // 4 < NQ:
            idx = quad_cols(d // 4)
            val = np.where(idx >= 0, 1.0, 0.0)
            w2fwd[:e - s, (d // 4) * 128:(d // 4 + 1) * 128] = (
                W2m[s:e][:, np.maximum(idx, 0)] * val)

    # completion weights, replicated on all 128 partitions:
    #   w1cF[*, i, u] = W1m[i-1, S[i]+u]   (fresh z col, on the chain)
    w1f = np.zeros((D, MMAX), np.float32)
    for i in range(1, D):
        s, e = int(_S[i]), int(_E[i])
        w1f[i, :e - s] = W1m[i - 1, s:e]
    w1cF = np.broadcast_to(w1f.reshape(1, -1), (128, D * MMAX))

    W1mp = np.zeros((D, H + MMAX), np.float32)
    W1mp[:, :H] = W1m
    import ml_dtypes
    bf = ml_dtypes.bfloat16
    return {
        "w1m": W1mp.astype(bf),
        "w1z": np.zeros((1, MMAX), bf),
        "w2q": w2q.astype(bf),
        "w2src": w2src.astype(bf),
        "w2fwd": w2fwd.astype(bf),
        "wct": np.ascontiguousarray(Wcs.T).astype(bf),         # (256, 512)
        "b1r": np.ascontiguousarray(b1s_[None, :]).astype(bf),  # (1, 512)
        "w1cF": np.ascontiguousarray(w1cF, np.float32),
        "ident": np.eye(128, dtype=np.float32),
        "womp": womp.astype(bf),
        "b2q": b2q,
        "bo2": np.ascontiguousarray(bo[None, :], np.float32),  # (1, 128)
        "ones": np.ones((1, 128), np.float32),
        "onesb": np.ones((1, 128), bf),
        "zz": np.zeros((D, BS), bf),
    }


_NC_CACHE = {}


def _build():
    if "nc" in _NC_CACHE:
        return _NC_CACHE["nc"]
    from contextlib import ExitStack

    import concourse.mybir as mybir
    import concourse.tile as tile
    from concourse import bacc

    f32 = mybir.dt.float32
    f32r = mybir.dt.float32r
    bf16 = mybir.dt.bfloat16
    AF = mybir.ActivationFunctionType
    OP = mybir.AluOpType
    AX = mybir.AxisListType

    def r(ap):
        # fp32 operands pass through: the fast matmuls all run in bf16
        # (float32r needs producer-side rounding the BIR verifier enforces)
        return ap

    # All ACT funcs used here (exp, ln, relu, identity) live in the
    # "natural_log_exp_and_others" table.  The greedy table-selection pass
    # otherwise ping-pongs exp->ln between single-func tables, inserting
    # ~256 table loads.  Keep dict order (index == act_func_set_id) but
    # blank every other table so selection sticks to the combined one.
    import concourse.bacc as bacc_mod
    _orig_tables = bacc_mod.get_activation_tables

    def _one_table(arch):
        tabs = _orig_tables(arch)
        return {k: (v if k == "natural_log_exp_and_others" else set())
                for k, v in tabs.items()}

    bacc_mod.get_activation_tables = _one_table

    nc = bacc.Bacc("TRN2", target_bir_lowering=False)

    ctxT_d = nc.dram_tensor("ctxT", [CTX, BS], bf16, kind="ExternalInput")
    epsB_d = nc.dram_tensor("epsB", [128, 8 * D], f32, kind="ExternalInput")
    w1m_d = nc.dram_tensor("w1m", [D, H + MMAX], bf16, kind="ExternalInput")
    w1z_d = nc.dram_tensor("w1z", [1, MMAX], bf16, kind="ExternalInput")
    w2q_d = nc.dram_tensor("w2q", [H, 16 * 128], bf16, kind="ExternalInput")
    wct_d = nc.dram_tensor("wct", [CTX, H], bf16, kind="ExternalInput")
    b1r_d = nc.dram_tensor("b1r", [1, H], bf16, kind="ExternalInput")
    w1f_d = nc.dram_tensor("w1cF", [128, D * MMAX], f32, kind="ExternalInput")
    id_d = nc.dram_tensor("ident", [128, 128], f32, kind="ExternalInput")
    w2s_d = nc.dram_tensor("w2src", [MMAX, D * 128], bf16, kind="ExternalInput")
    w2f_d = nc.dram_tensor("w2fwd", [MMAX, 16 * 128], bf16, kind="ExternalInput")
    womp_d = nc.dram_tensor("womp", [128, D * 2 * D], bf16, kind="ExternalInput")
    b2q_d = nc.dram_tensor("b2q", [128, 16], f32, kind="ExternalInput")
    bo2_d = nc.dram_tensor("bo2", [1, 2 * D], f32, kind="ExternalInput")
    ones_d = nc.dram_tensor("ones", [1, 128], f32, kind="ExternalInput")
    onesb_d = nc.dram_tensor("onesb", [1, 128], bf16, kind="ExternalInput")
    zz_d = nc.dram_tensor("zz", [D, BS], bf16, kind="ExternalInput")

    # outputs, batch-major (BS, D); rows r = h*512 + ch*128 + p
    zo_d = nc.dram_tensor("zo", [BS, D], f32, kind="ExternalOutput")
    mo_d = nc.dram_tensor("mo", [BS, D], f32, kind="ExternalOutput")
    so_d = nc.dram_tensor("so", [BS, D], f32, kind="ExternalOutput")

    with tile.TileContext(nc) as tc, ExitStack() as ctx:
        const = ctx.enter_context(tc.tile_pool(name="const", bufs=1))
        work = ctx.enter_context(tc.tile_pool(name="work", bufs=4))
        pout = ctx.enter_context(tc.tile_pool(name="pout", bufs=1, space="PSUM"))
        pscr = ctx.enter_context(tc.tile_pool(name="pscr", bufs=2, space="PSUM"))
        pzt = ctx.enter_context(tc.tile_pool(name="pzt", bufs=2, space="PSUM"))

        # ---- constants / state ----
        w1m = const.tile([D, H + MMAX], bf16)
        nc.sync.dma_start(w1m[:, :], w1m_d[:, :])
        w1z = const.tile([1, MMAX], bf16)
        nc.sync.dma_start(w1z[:, :], w1z_d[:, :])
        w2q = [const.tile([128, 16 * 128], bf16, name=f"w2q{g}") for g in range(4)]
        for g in range(4):
            nc.sync.dma_start(w2q[g][:, :], w2q_d[g * 128:(g + 1) * 128, :])
        wct = [const.tile([128, H], bf16, name=f"wct{k}") for k in range(2)]
        for k in range(2):
            nc.sync.dma_start(wct[k][:, :], wct_d[k * 128:(k + 1) * 128, :])
        ctxT = [const.tile([128, BS], bf16, name=f"ctxTs{k}") for k in range(2)]
        for k in range(2):
            nc.sync.dma_start(ctxT[k][:, :], ctxT_d[k * 128:(k + 1) * 128, :])
        b1r = const.tile([1, H], bf16)
        nc.sync.dma_start(b1r[:, :], b1r_d[:, :])
        w1cF = const.tile([128, D * MMAX], f32)
        nc.sync.dma_start(w1cF[:, :], w1f_d[:, :])
        ident = const.tile([128, 128], f32)
        nc.sync.dma_start(ident[:, :], id_d[:, :])
        w2src = const.tile([MMAX, D * 128], bf16)
        nc.sync.dma_start(w2src[:, :], w2s_d[:, :])
        w2fwd = const.tile([MMAX, 16 * 128], bf16)
        nc.sync.dma_start(w2fwd[:, :], w2f_d[:, :])
        womp = const.tile([128, D * 2 * D], bf16)
        nc.sync.dma_start(womp[:, :], womp_d[:, :])
        b2q = const.tile([128, 16], f32)
        nc.sync.dma_start(b2q[:, :], b2q_d[:, :])
        bo2 = const.tile([1, 2 * D], f32)
        nc.sync.dma_start(bo2[:, :], bo2_d[:, :])
        ones = const.tile([1, 128], f32)
        nc.sync.dma_start(ones[:, :], ones_d[:, :])
        onesb = const.tile([1, 128], bf16)
        nc.sync.dma_start(onesb[:, :], onesb_d[:, :])
        epsB = const.tile([128, 8 * D], f32)
        nc.sync.dma_start(epsB[:, :], epsB_d[:, :])
        zT = const.tile([D, BS], bf16)
        nc.sync.dma_start(zT[:, :], zz_d[:, :])

        h1g = [const.tile([128, BS], bf16, name=f"h1g{g}") for g in range(4)]
        scB = const.tile([128, 8 * D], f32)
        zB = const.tile([128, 8 * D], f32)
        ctxB = const.tile([128, 8 * (H + MMAX)], f32)
        nc.vector.memset(zB[:, :], 0.0)
        nc.vector.memset(ctxB[:, :], 0.0)

        # persistent transposed output accumulators: [batch 128, 4ch x 128 out]
        outp = [pout.tile([128, 4 * 128], f32, name=f"outp{h}") for h in range(2)]

        def ov(h, ch):            # (128, 128) chunk view of the accumulator
            return outp[h][:, ch * 128:(ch + 1) * 128]

        def ocol(h, o):           # (128, 4) strided column view, output o
            return outp[h][:, :].rearrange("p (c o) -> p c o", c=4)[:, :, o]

        def bcolB(t, i):          # (128, 8) strided column, both halves
            return t[:, :].rearrange("p (g d) -> p g d", g=8)[:, :, i]

        def bcol(t, h, i):        # (128, 4) strided column of half h
            return t[:, h * 4 * D:(h + 1) * 4 * D].rearrange(
                "p (c d) -> p c d", c=4)[:, :, i]

        # ---- init: bias rows of the output accumulator ----
        for h in range(2):
            for ch in range(4):
                nc.tensor.matmul(ov(h, ch), ones[0:1, :], bo2[0:1, :],
                                 start=True, stop=True)

        # ---- init: batch-land ctx projection + b1 (ctxB[h][p, c*H+u]) ----
        for h in range(2):
            for c in range(4):
                pc = pscr.tile([128, H], f32, tag="p2", name=f"pc{h}_{c}")
                for k in range(2):
                    cstat = ctxT[k][:, h * NH + c * 128:h * NH + (c + 1) * 128]
                    nc.tensor.matmul(pc[:, :], r(cstat),
                                     r(wct[k][:, :]), start=(k == 0),
                                     stop=False)
                nc.tensor.matmul(pc[:, :], onesb[0:1, :],
                                 b1r[0:1, :], start=False, stop=True)
                gc = h * 4 + c
                nc.scalar.activation(
                    ctxB[:, gc * (H + MMAX):gc * (H + MMAX) + H], pc[:, :],
                    AF.Identity)

        def extract(i):
            # softplus(x) = ln(exp(x) + 1)  (no softplus table on HW)
            for h in range(2):
                nc.scalar.activation(bcol(scB, h, i), ocol(h, D + i), AF.Exp)
                nc.scalar.activation(bcol(scB, h, i), bcol(scB, h, i),
                                     AF.Ln, bias=1.0)
                nc.vector.tensor_tensor(bcol(zB, h, i), bcol(scB, h, i),
                                        bcol(epsB, h, i), OP.mult)
                nc.vector.tensor_tensor(bcol(zB, h, i), bcol(zB, h, i),
                                        ocol(h, i), OP.add)
            # z_i to unit-land: PE transpose (128,8)->(8,128), then one
            # contiguous PSUM->SBUF DMA into zT row i (SP queue).
            # ~4us total latency, consumed only at step i+KC+1.
            with tc.high_priority(offset=-300):
                ztp = pzt.tile([8, 128], f32, tag="ztp", name=f"ztp{i}")
                nc.tensor.matmul(r(ztp[:, :]), r(bcolB(zB, i)),
                                 r(ident[:, :]), is_transpose=True,
                                 start=True, stop=True)
                zsb = work.tile([8, 128], bf16, tag="zsb", name=f"zsb{i}")
                nc.vector.tensor_copy(zsb[:, :], ztp[:, :])
                nc.sync.dma_start(
                    zT[i:i + 1, :].rearrange("a (g p) -> a g p", g=8),
                    zsb[:, :])

        extract(0)

        h1n_t = {}                 # live per-degree h1 tiles
        p2q = [None, None]         # current quad's L2 accumulator per half
        h2q = [None, None]         # current quad's relu'd h2 rows per half
        for i in range(1, D):
            s, e = int(_S[i]), int(_E[i])
            m = e - s
            g_, r0_ = s // 128, s % 128
            q, sd = (i - 1) // 4, (i - 1) % 4
            ps = int(_S[i - 1])    # h1g rows < ps are >= 2 steps old
            nz = max(0, i - KC)    # zT rows entering via the matmul
            j0 = max(0, i - KC)    # first z col entering via completion
            # --- completions, batch-land, both halves per op.  The old
            #     z col (i-2) + ctx are off the chain; only the fresh col
            #     (i-1) product and one add separate z_{i-1} from p1. ---
            def colv(t, j):       # (128, 8, MMAX) broadcast of col j
                return t[:, :].rearrange("p (g d) -> p g d", g=8)[
                    :, :, j:j + 1].broadcast_to((128, 8, MMAX))

            def wv9(t):           # (128, 8, MMAX) broadcast of a weight row
                return t[:, MMAX * i:MMAX * (i + 1)].rearrange(
                    "p (o u) -> p o u", o=1).broadcast_to((128, 8, MMAX))

            ctxv = ctxB[:, :].rearrange(
                "p (g u) -> p g u", g=8)[:, :, s:s + MMAX]
            cred = work.tile([128, 8 * MMAX], f32, tag="cred",
                             name=f"cred_{i}")
            credv = cred[:, :].rearrange("p (g u) -> p g u", g=8)
            for h in range(2):
                hs = slice(4 * h, 4 * (h + 1))
                eng = nc.gpsimd if h == 0 else nc.vector
                eng.tensor_tensor(credv[:, hs, :],
                                  colv(zB, i - 1)[:, hs, :],
                                  wv9(w1cF)[:, hs, :], OP.mult)
                eng.tensor_tensor(credv[:, hs, :], credv[:, hs, :],
                                  ctxv[:, hs, :], OP.add)
            for h in range(2):
                cs = slice(h * NH, (h + 1) * NH)
                # --- L1: lagged z-matmul (rows <= i-KC-1), off the chain ---
                p1 = pscr.tile([MMAX, NH], f32, tag="p1", name=f"p1_{i}_{h}")
                if nz > 0:
                    nc.tensor.matmul(p1[0:MMAX, :], r(w1m[0:nz, s:s + MMAX]),
                                     r(zT[0:nz, cs]), start=True, stop=False)
                else:
                    nc.tensor.matmul(p1[0:MMAX, :], r(w1z[0:1, 0:MMAX]),
                                     r(zT[0:1, cs]), start=True, stop=False)
                # --- transpose-accumulate completions into the p1 group ---
                for c in range(4):
                    nc.tensor.matmul(
                        r(p1[0:MMAX, 128 * c:128 * (c + 1)]),
                        r(credv[:, h * 4 + c, :]), r(ident[:, :]),
                        is_transpose=True, start=False, stop=(c == 3),
                        skip_group_check=(c != 3))
                # --- h1 relu (Pool, bf16 out) + assembly into h1g (Pool q) ---
                h1n = work.tile([MMAX, NH], bf16, tag="h1n", name=f"h1n_{i}_{h}")
                if h == 0:
                    nc.scalar.activation(h1n[0:m, :], p1[0:m, :], AF.Relu)
                else:
                    nc.vector.tensor_scalar_max(h1n[0:m, :], p1[0:m, :], 0.0)
                with tc.high_priority(offset=-300):
                    eng = nc.sync if h == 0 else nc.gpsimd
                    eng.dma_start(h1g[g_][r0_:r0_ + m, cs], h1n[0:m, :])
                h1n_t[(i, h)] = h1n
                # --- L2, quad-grained: at each quad start accumulate all
                #     h1 degs <= i into the (128, 512) quad psum; slots 1-3
                #     add only their own degree's contribution. ---
                qsl = slice(q * 128, (q + 1) * 128)
                if sd == 0:
                    p2 = pscr.tile([128, NH], f32, tag="p2", name=f"p2_{i}_{h}")
                    p2q[h] = p2
                    h2q[h] = work.tile([128, NH], bf16, tag="h2n",
                                       name=f"h2n_{i}_{h}")
                    ngg = ps // 128
                    nmm = ngg + (1 if ps % 128 else 0) + (1 if i >= 2 else 0) + 1
                    kk = 0
                    for gg in range(ngg):
                        nc.tensor.matmul(p2[:, :], w2q[gg][:, qsl],
                                         h1g[gg][:, cs],
                                         start=(kk == 0), stop=False)
                        kk += 1
                    if ps % 128:
                        nc.tensor.matmul(p2[:, :], w2q[ngg][0:ps % 128, qsl],
                                         h1g[ngg][0:ps % 128, cs],
                                         start=(kk == 0), stop=False)
                        kk += 1
                    if i >= 2:
                        pm = int(_E[i - 1]) - ps
                        nc.tensor.matmul(p2[:, :],
                                         w2fwd[0:pm, q * 128:(q + 1) * 128],
                                         h1n_t[(i - 1, h)][0:pm, :],
                                         start=(kk == 0), stop=False)
                        kk += 1
                    for cb in range(2):
                        csl = slice(cb * NH // 2, (cb + 1) * NH // 2)
                        nc.tensor.matmul(p2[:, csl],
                                         w2src[0:m, i * 128:(i + 1) * 128],
                                         h1n[0:m, csl],
                                         start=(kk == 0 and cb == 0),
                                         stop=(cb == 1))
                else:
                    for cb in range(2):
                        csl = slice(cb * NH // 2, (cb + 1) * NH // 2)
                        nc.tensor.matmul(p2q[h][:, csl],
                                         w2src[0:m, i * 128:(i + 1) * 128],
                                         h1n[0:m, csl], start=False, stop=True,
                                         skip_group_check=True)
                # --- h2 relu of this degree's 32-aligned quad rows,
                #     split across ACT and DVE to shorten the chain ---
                h2n = h2q[h]
                o_ = 32 * sd
                if h == 0:
                    nc.scalar.activation(h2n[o_:o_ + m, :],
                                         p2q[h][o_:o_ + m, :], AF.Relu,
                                         bias=b2q[o_:o_ + m, q:q + 1])
                else:
                    nc.vector.tensor_scalar(h2n[o_:o_ + m, :],
                                            p2q[h][o_:o_ + m, :],
                                            b2q[o_:o_ + m, q:q + 1], 0.0,
                                            OP.add, OP.max)
                # --- L3: masked accumulate (cols i..63, D+i..2D-1 only) ---
                wb = womp[32 * sd:32 * sd + m, i * 2 * D:(i + 1) * 2 * D]
                for ch in range(4):
                    for t in range(2):
                        nc.tensor.matmul(
                            ov(h, ch)[:, t * D + i:(t + 1) * D],
                            h2n[32 * sd:32 * sd + m, ch * 128:(ch + 1) * 128],
                            wb[:, t * D + i:(t + 1) * D],
                            start=False, stop=True, skip_group_check=True,
                            tile_position=(32 * sd, 0))
            extract(i)

        # ---- outputs (batch-major rows r = h*512 + ch*128 + p) ----
        for h in range(2):
            dst = slice(h * NH, (h + 1) * NH)
            for name_d, t in ((zo_d, zB), (so_d, scB)):
                nc.sync.dma_start(
                    name_d[dst, :].rearrange("(c p) d -> p c d", c=4),
                    t[:, h * 4 * D:(h + 1) * 4 * D].rearrange(
                        "p (c d) -> p c d", c=4))
            # mu out of the PSUM accumulator via one SBUF bounce
            muB = work.tile([128, 4 * D], f32, tag="muB", name=f"muB_{h}")
            nc.scalar.activation(
                muB[:, :].rearrange("p (c d) -> p c d", c=4),
                outp[h][:, :].rearrange("p (c o) -> p c o", c=4)[:, :, 0:D],
                AF.Identity)
            nc.sync.dma_start(
                mo_d[dst, :].rearrange("(c p) d -> p c d", c=4),
                muB[:, :].rearrange("p (c d) -> p c d", c=4))

    nc.compile()
    _NC_CACHE["nc"] = nc
    return nc


def make_in_maps(context, eps, W1, b1, Wc, W2, b2, Wo, bo):
    context = np.asarray(context, np.float32)
    eps = np.asarray(eps, np.float32)
    wd = _host_weights(np.asarray(W1, np.float32), np.asarray(b1, np.float32),
                       np.asarray(Wc, np.float32), np.asarray(W2, np.float32),
                       np.asarray(b2, np.float32), np.asarray(Wo, np.float32),
                       np.asarray(bo, np.float32))

    in_maps = []
    for c in range(NCORES):
        sl = slice(c * BS, (c + 1) * BS)
        ctx_s = context[sl]                       # (1024, 256)
        eps_s = eps[sl]                           # (1024, 64)
        im = dict(wd)
        im["ctxT"] = np.ascontiguousarray(ctx_s.T).astype(
            wd["wct"].dtype)                                  # (256, 1024)
        im["epsB"] = np.ascontiguousarray(
            eps_s.reshape(2, 4, 128, D).transpose(2, 0, 1, 3).reshape(
                128, 8 * D))
        in_maps.append(im)
    return in_maps


def assemble_outputs(results):
    z = np.concatenate([r["zo"] for r in results], axis=0)
    mus = np.concatenate([r["mo"] for r in results], axis=0)
    scales = np.concatenate([r["so"] for r in results], axis=0)
    return z, mus, scales


def kernel(context, eps, W1, b1, Wc, W2, b2, Wo, bo, _trace=False):
    from concourse.bass_utils import run_bass_kernel_spmd

    in_maps = make_in_maps(context, eps, W1, b1, Wc, W2, b2, Wo, bo)
    nc = _build()
    res = run_bass_kernel_spmd(nc, in_maps, core_ids=list(range(NCORES)),
                               trace=_trace)
    if _trace:
        kernel.last_exec_time_ns = res.exec_time_ns
        kernel.last_results = res
    return assemble_outputs(res.results)
